# revision 19
# baseline (speedup 1.0000x reference)
"""ON-LSTM cell fused kernel for Trainium2, data-parallel over 8 NeuronCores.

Reference computation (per batch row b):
    gates = x @ W_ih.T + b_ih + hx @ W_hh.T + b_hh          # [B, 4128]
    cin_raw = gates[:, :16]; cfg_raw = gates[:, 16:32]
    rest = gates[:, 32:].reshape(B, 64, 64)
    outgate = sigmoid(rest[:, 0:16]); cell = tanh(rest[:, 16:32])
    ingate = sigmoid(rest[:, 32:48]); forgetgate = sigmoid(rest[:, 48:64])
    cingate = 1 - cumsum(softmax(cin_raw)); cforgetgate = cumsum(softmax(cfg_raw))
    ov = cingate*cforgetgate
    f = forgetgate*ov + (cforgetgate - ov); i = ingate*ov + (cingate - ov)
    cy = f*cx + i*cell; hy = outgate*tanh(cy)
    dcf = 1 - sum(cforgetgate)/16; dci = sum(cingate)/16

Device strategy (per core, 1024 batch rows):
  - Host pre-transposes/concats activations (xT [2048, B]) and weights
    (WT [2048, 4128]) with a gate-column permutation so each chunk's four
    gates are contiguous: new col ch*256 + [cell|out|in|fgt]*64, then
    cin raw at 4096:4112, cfg raw at 4112:4128.
  - gatesT accumulated in PSUM via fp32r matmuls (full PE rate at N=512),
    16 K-tiles, all 8 PSUM banks = 8 batch tiles of 128; each PSUM tile is
    consumed immediately by fused gate math so gates never hit SBUF/DRAM.
  - cumsoftmax via ACT Exp(+accum sum) + DVE tensor_tensor_scan (hw cumsum).
"""

import hashlib
import os
import shutil

import numpy as np

import concourse.bass as bass
import concourse.mybir as mybir
import concourse.tile as tile
from concourse import bass2jax, bass_utils
from concourse.bass_utils import run_bass_kernel_spmd

_NEFF_CACHE = os.path.expanduser("~/.cache/bass_neff")


def _patch_toolchain():
    """Speed up iteration: skip walrus' in-compile BIR simulation (the stock
    neuronx-cc pipeline runs with --enable-birsim=false too) and cache NEFFs
    by BIR hash so identical kernels skip the multi-minute walrus run."""
    if getattr(bass_utils, "_onlstm_patched", False):
        return
    bass_utils._onlstm_patched = True

    orig_run = bass_utils.run_command

    def run_nobirsim(argv, **kw):
        argv = [
            "--enable-birsim=false" if a == "--enable-birsim=true" else a
            for a in argv
        ]
        return orig_run(argv, **kw)

    bass_utils.run_command = run_nobirsim

    orig_compile = bass_utils.compile_bir_kernel

    def cached_compile(bir_json, tmpdir, neff_name="file.neff"):
        raw = bir_json if isinstance(bir_json, bytes) else bir_json.encode()
        try:
            import orjson

            def strip(o):
                if isinstance(o, dict):
                    return {
                        k: strip(v)
                        for k, v in o.items()
                        if k not in ("debug", "debug_info", "ant_debug")
                    }
                if isinstance(o, list):
                    return [strip(v) for v in o]
                return o

            # key on debug-stripped BIR so the cache is path-independent
            key_bytes = orjson.dumps(strip(orjson.loads(raw)))
        except Exception:
            key_bytes = raw
        h = hashlib.sha256(key_bytes).hexdigest()[:32]
        cpath = os.path.join(_NEFF_CACHE, f"{h}.neff")
        dst = os.path.join(tmpdir, neff_name)
        if os.path.exists(cpath):
            shutil.copy(cpath, dst)
            return dst
        out = orig_compile(bir_json, tmpdir, neff_name)
        os.makedirs(_NEFF_CACHE, exist_ok=True)
        tmp = cpath + ".tmp"
        shutil.copy(out, tmp)
        os.replace(tmp, cpath)
        return out

    bass_utils.compile_bir_kernel = cached_compile
    bass2jax.compile_bir_kernel = cached_compile

F32 = mybir.dt.float32
F32R = mybir.dt.float32r
FP16 = mybir.dt.float16
AF = mybir.ActivationFunctionType
ALU = mybir.AluOpType

B, IN, H, CHUNK, NCH = 8192, 1024, 1024, 64, 16
GATE = 4 * H + 2 * NCH  # 4128
NCORES = 8
BC = B // NCORES  # 1024 rows per core
K = IN + H  # 2048 contraction
KT = K // 128  # 16 k-tiles
MT = BC // 128  # 8 batch tiles per core
NT = (4 * H) // 512  # 8 big n-tiles over the 4096 permuted gate cols


def _gate_perm():
    """new_col -> old_col so that gates_new = gates_old[:, perm]."""
    perm = np.empty(GATE, dtype=np.int64)
    for ch in range(NCH):
        base = ch * 4 * CHUNK
        j = np.arange(CHUNK)
        perm[base + 0 * CHUNK + j] = 32 + 1 * H + ch * CHUNK + j  # cell
        perm[base + 1 * CHUNK + j] = 32 + 0 * H + ch * CHUNK + j  # outgate
        perm[base + 2 * CHUNK + j] = 32 + 2 * H + ch * CHUNK + j  # ingate
        perm[base + 3 * CHUNK + j] = 32 + 3 * H + ch * CHUNK + j  # forgetgate
    perm[4 * H : 4 * H + NCH] = np.arange(NCH)  # cin raw
    perm[4 * H + NCH :] = NCH + np.arange(NCH)  # cfg raw
    return perm


def _split_waits(nc):
    """This walrus build caps sem-waits per instruction (1 for NoOp/Drain
    control encodings, 2 for compute/DMA). Move excess waits onto preceding
    single-wait NoOps on the same engine."""
    fn = nc.m.functions[0]
    for blk in fn.blocks:
        newlist = []
        for ins in blk.instructions:
            si = ins.sync_info
            maxw = 1
            if si is not None and len(si.on_wait) > maxw:
                waits = list(si.on_wait)
                while len(waits) > maxw:
                    chunk, waits = waits[:1], waits[1:]
                    nop = mybir.InstNoOp(
                        name=nc.get_next_instruction_name(), ins=[], outs=[]
                    )
                    nop.engine = ins.engine
                    nop.sync_info = mybir.SyncInfo(on_wait=chunk, on_update=[])
                    nc.register_instruction(nop)
                    newlist.append(nop)
                ins.sync_info = mybir.SyncInfo(
                    on_wait=waits, on_update=list(si.on_update)
                )
            newlist.append(ins)
        blk.instructions[:] = newlist


def _build_bass():
    nc = bass.Bass()
    xt = nc.dram_tensor("xt", [K, BC], FP16, kind="ExternalInput")
    wt = nc.dram_tensor("wt", [K, GATE], FP16, kind="ExternalInput")
    biasd = nc.dram_tensor("biasd", [GATE], F32, kind="ExternalInput")
    cxd = nc.dram_tensor("cxd", [BC, H], F32, kind="ExternalInput")
    hyd = nc.dram_tensor("hyd", [BC, H], F32, kind="ExternalOutput")
    cyd = nc.dram_tensor("cyd", [BC, H], F32, kind="ExternalOutput")
    dcfd = nc.dram_tensor("dcfd", [BC], F32, kind="ExternalOutput")
    dcid = nc.dram_tensor("dcid", [BC], F32, kind="ExternalOutput")

    with tile.TileContext(nc) as tc:
        with (
            tc.tile_pool(name="big", bufs=1) as big,
            tc.tile_pool(name="wpool", bufs=10) as wpool,
            tc.tile_pool(name="work", bufs=3) as work,
            tc.tile_pool(name="small", bufs=3) as small,
            tc.tile_pool(name="psum", bufs=8, space="PSUM") as psum,
        ):
            # ---- resident tiles ----
            # All resident loads ride the GpSimd SWDGE queues so the SP HWDGE
            # queue is free for the W stream from t=0. Order: wtail first
            # (phase A blocks on it), then xt k-tiles, then bias/cx (needed
            # only once gate math starts).
            wtail_sb = big.tile([128, KT * 32], FP16)  # cin/cfg weight cols
            for k in range(KT):
                nc.gpsimd.dma_start(
                    out=wtail_sb[:, k * 32 : (k + 1) * 32],
                    in_=wt[k * 128 : (k + 1) * 128, 4 * H :],
                )
            xt_sb = big.tile([128, KT * BC], FP16)  # k-tile k at cols [k*BC,(k+1)*BC)
            for k in range(KT):
                nc.sync.dma_start(
                    out=xt_sb[:, k * BC : (k + 1) * BC],
                    in_=xt[k * 128 : (k + 1) * 128, :],
                )
            bias_sb = big.tile([128, GATE], F32)
            nc.gpsimd.dma_start(
                out=bias_sb,
                in_=bass.AP(tensor=biasd, offset=0, ap=[[0, 128], [1, GATE]]),
            )
            cx_sb = big.tile([128, MT * H], F32)  # m-tile m at cols [m*H,(m+1)*H)
            for m in range(MT):
                nc.gpsimd.dma_start(
                    out=cx_sb[:, m * H : (m + 1) * H],
                    in_=cxd[m * 128 : (m + 1) * 128, :],
                )
            # per-m chunk-gate vectors kept across phase B; icf_all interleaves
            # (cingate-ov, cforgetgate-ov) per chunk so phase B can broadcast
            # both with a single 4D AP
            ov_all = big.tile([128, MT * NCH], F32)
            icf_all = big.tile([128, MT * NCH * 2], F32)
            icf = icf_all.rearrange("p (m c w) -> p m c w", m=MT, c=NCH)

            def lhs(k, m):
                off = k * BC + m * 128
                return xt_sb[:, off : off + 128]

            # ---- phase A: cin/cfg columns, cumsoftmax machinery ----
            for m in range(MT):
                ps_a = psum.tile([128, 32], F32, tag="ps")
                for k in range(KT):
                    nc.tensor.matmul(
                        ps_a,
                        lhs(k, m),
                        wtail_sb[:, k * 32 : (k + 1) * 32],
                        start=(k == 0),
                        stop=(k == KT - 1),
                    )
                raw = small.tile([128, 32], F32)
                nc.vector.tensor_add(raw, ps_a, bias_sb[:, 4 * H :])
                expt = small.tile([128, 32], F32)
                sums = small.tile([128, 2], F32)
                nc.scalar.activation(
                    out=expt[:, :16], in_=raw[:, :16], func=AF.Exp,
                    accum_out=sums[:, 0:1],
                )
                nc.scalar.activation(
                    out=expt[:, 16:], in_=raw[:, 16:], func=AF.Exp,
                    accum_out=sums[:, 1:2],
                )
                rec = small.tile([128, 2], F32)
                nc.vector.reciprocal(out=rec, in_=sums)
                nrec = small.tile([128, 1], F32)
                nc.scalar.mul(out=nrec, in_=rec[:, 0:1], mul=-1.0)
                scan = small.tile([128, 32], F32)
                nc.vector.tensor_tensor_scan(
                    out=scan[:, :16], data0=expt[:, :16], data1=expt[:, :16],
                    initial=0.0, op0=ALU.add, op1=ALU.bypass,
                )
                nc.vector.tensor_tensor_scan(
                    out=scan[:, 16:], data0=expt[:, 16:], data1=expt[:, 16:],
                    initial=0.0, op0=ALU.add, op1=ALU.bypass,
                )
                ci = small.tile([128, NCH], F32)
                nc.vector.tensor_scalar(
                    out=ci, in0=scan[:, :16], scalar1=nrec[:, 0:1], scalar2=1.0,
                    op0=ALU.mult, op1=ALU.add,
                )
                cf = small.tile([128, NCH], F32)
                nc.vector.tensor_scalar_mul(out=cf, in0=scan[:, 16:], scalar1=rec[:, 1:2])
                msl = slice(m * NCH, (m + 1) * NCH)
                nc.vector.tensor_mul(ov_all[:, msl], ci, cf)
                nc.vector.tensor_sub(icf[:, m, :, 0], ci, ov_all[:, msl])
                nc.vector.tensor_sub(icf[:, m, :, 1], cf, ov_all[:, msl])
                # distances
                dsum = small.tile([128, 2], F32)
                nc.vector.reduce_sum(out=dsum[:, 0:1], in_=cf, axis=mybir.AxisListType.X)
                nc.vector.reduce_sum(out=dsum[:, 1:2], in_=ci, axis=mybir.AxisListType.X)
                dout = small.tile([128, 2], F32)
                nc.scalar.activation(
                    out=dout[:, 0:1], in_=dsum[:, 0:1], func=AF.Copy,
                    scale=-1.0 / NCH, bias=1.0,
                )
                nc.scalar.activation(
                    out=dout[:, 1:2], in_=dsum[:, 1:2], func=AF.Copy,
                    scale=1.0 / NCH, bias=0.0,
                )
                nc.gpsimd.dma_start(
                    out=dcfd.rearrange("(m p) -> m p", p=128)[m][:, None],
                    in_=dout[:, 0:1],
                )
                nc.gpsimd.dma_start(
                    out=dcid.rearrange("(m p) -> m p", p=128)[m][:, None],
                    in_=dout[:, 1:2],
                )

            # ---- phase B: main gates, 512 cols (= 2 chunks) per n-tile ----
            cx4 = cx_sb.rearrange("p (m c j) -> p m c j", m=MT, c=NCH)
            cy3 = cyd.rearrange("b (c j) -> b c j", c=NCH)
            hy3 = hyd.rearrange("b (c j) -> b c j", c=NCH)
            for n in range(NT):
                ps_tiles = [psum.tile([128, 512], F32, tag="ps", name=f"ps_{n}_{m}") for m in range(MT)]
                for k in range(KT):
                    wk = wpool.tile([128, 512], FP16, tag="w", name=f"w_{n}_{k}")
                    nc.sync.dma_start(
                        out=wk, in_=wt[k * 128 : (k + 1) * 128, n * 512 : (n + 1) * 512]
                    )
                    for m in range(MT):
                        nc.tensor.matmul(
                            ps_tiles[m],
                            lhs(k, m),
                            wk,
                            start=(k == 0),
                            stop=(k == KT - 1),
                        )
                for m in range(MT):
                    g = work.tile([128, 2, 256], F32, tag="g", name=f"g_{n}_{m}")
                    nc.vector.tensor_add(
                        g,
                        ps_tiles[m].rearrange("p (c q) -> p c q", c=2),
                        bias_sb[:, n * 512 : (n + 1) * 512].rearrange(
                            "p (c q) -> p c q", c=2
                        ),
                    )
                    act = work.tile([128, 2, 256], F32, tag="act", name=f"act_{n}_{m}")
                    nc.scalar.activation(
                        out=act[:, :, 64:256], in_=g[:, :, 64:256], func=AF.Sigmoid
                    )
                    nc.scalar.activation(
                        out=act[:, :, 0:64], in_=g[:, :, 0:64], func=AF.Tanh
                    )
                    ch = slice(m * NCH + 2 * n, m * NCH + 2 * n + 2)
                    ovb4 = ov_all[:, ch][:, :, None, None].broadcast_to(
                        [128, 2, 2, 64]
                    )
                    icfb4 = icf[:, m, 2 * n : 2 * n + 2, :][
                        :, :, :, None
                    ].broadcast_to([128, 2, 2, 64])
                    act4 = act[:, :, 128:256].rearrange(
                        "p c (w j) -> p c w j", w=2
                    )  # [.., 0, :]=ingate, [.., 1, :]=forgetgate
                    fi = work.tile([128, 2, 2, 64], F32, tag="fi", name=f"fi_{n}_{m}")
                    nc.vector.tensor_mul(fi, act4, ovb4)
                    nc.vector.tensor_add(fi, fi, icfb4)
                    # i_final*cell and f_final*cx on GpSimd to unload DVE
                    nc.gpsimd.tensor_mul(fi[:, :, 0, :], fi[:, :, 0, :], act[:, :, 0:64])
                    nc.gpsimd.tensor_mul(
                        fi[:, :, 1, :], fi[:, :, 1, :], cx4[:, m, 2 * n : 2 * n + 2, :]
                    )
                    cyt = work.tile([128, 2, 64], F32, tag="cyt", name=f"cyt_{n}_{m}")
                    nc.vector.tensor_add(cyt, fi[:, :, 0, :], fi[:, :, 1, :])
                    nc.sync.dma_start(
                        out=cy3[m * 128 : (m + 1) * 128, 2 * n : 2 * n + 2, :],
                        in_=cyt,
                    )
                    th = work.tile([128, 2, 64], F32, tag="th", name=f"th_{n}_{m}")
                    nc.scalar.activation(out=th, in_=cyt, func=AF.Tanh)
                    nc.vector.tensor_mul(th, th, act[:, :, 64:128])
                    nc.scalar.dma_start(
                        out=hy3[m * 128 : (m + 1) * 128, 2 * n : 2 * n + 2, :],
                        in_=th,
                    )
    _split_waits(nc)
    return nc


_PERM = _gate_perm()
LAST_RESULT = None


def kernel(x, hx, cx, W_ih, b_ih, W_hh, b_hh):
    x = np.asarray(x, dtype=np.float32)
    hx = np.asarray(hx, dtype=np.float32)
    cx = np.asarray(cx, dtype=np.float32)
    Wcat = np.concatenate(
        [np.asarray(W_ih, np.float32), np.asarray(W_hh, np.float32)], axis=1
    )  # [GATE, K]
    wt_np = np.ascontiguousarray(Wcat[_PERM].T)  # [K, GATE]
    bias_np = np.ascontiguousarray(
        (np.asarray(b_ih, np.float32) + np.asarray(b_hh, np.float32))[_PERM]
    )
    xcatT = np.concatenate([x, hx], axis=1).T  # [K, B]
    cx2 = np.ascontiguousarray(cx.reshape(B, H))
    wt_np = wt_np.astype(np.float16)
    xcatT = np.ascontiguousarray(xcatT.astype(np.float16))

    _patch_toolchain()
    nc = _build_bass()
    in_maps = []
    for c in range(NCORES):
        sl = slice(c * BC, (c + 1) * BC)
        in_maps.append(
            {
                "xt": np.ascontiguousarray(xcatT[:, sl]),
                "wt": wt_np,
                "biasd": bias_np,
                "cxd": cx2[sl],
            }
        )
    global LAST_RESULT
    LAST_RESULT = run_bass_kernel_spmd(nc, in_maps, list(range(NCORES)))
    res = LAST_RESULT.results
    hy = np.concatenate([res[c]["hyd"] for c in range(NCORES)], axis=0)
    cy = np.concatenate([res[c]["cyd"] for c in range(NCORES)], axis=0).reshape(
        B, NCH, CHUNK
    )
    dcf = np.concatenate([res[c]["dcfd"] for c in range(NCORES)], axis=0)
    dci = np.concatenate([res[c]["dcid"] for c in range(NCORES)], axis=0)
    return hy, cy, dcf, dci


# revision 21
# speedup vs baseline: 1.0471x; 1.0471x over previous
"""ON-LSTM cell fused kernel for Trainium2, data-parallel over 8 NeuronCores.

Reference computation (per batch row b):
    gates = x @ W_ih.T + b_ih + hx @ W_hh.T + b_hh          # [B, 4128]
    cin_raw = gates[:, :16]; cfg_raw = gates[:, 16:32]
    rest = gates[:, 32:].reshape(B, 64, 64)
    outgate = sigmoid(rest[:, 0:16]); cell = tanh(rest[:, 16:32])
    ingate = sigmoid(rest[:, 32:48]); forgetgate = sigmoid(rest[:, 48:64])
    cingate = 1 - cumsum(softmax(cin_raw)); cforgetgate = cumsum(softmax(cfg_raw))
    ov = cingate*cforgetgate
    f = forgetgate*ov + (cforgetgate - ov); i = ingate*ov + (cingate - ov)
    cy = f*cx + i*cell; hy = outgate*tanh(cy)
    dcf = 1 - sum(cforgetgate)/16; dci = sum(cingate)/16

Device strategy (per core, 1024 batch rows):
  - Host pre-transposes/concats activations (xT [2048, B]) and weights
    (WT [2048, 4128]) with a gate-column permutation so each chunk's four
    gates are contiguous: new col ch*256 + [cell|out|in|fgt]*64, then
    cin raw at 4096:4112, cfg raw at 4112:4128.
  - gatesT accumulated in PSUM via fp32r matmuls (full PE rate at N=512),
    16 K-tiles, all 8 PSUM banks = 8 batch tiles of 128; each PSUM tile is
    consumed immediately by fused gate math so gates never hit SBUF/DRAM.
  - cumsoftmax via ACT Exp(+accum sum) + DVE tensor_tensor_scan (hw cumsum).
"""

import hashlib
import os
import shutil

import numpy as np

import concourse.bass as bass
import concourse.mybir as mybir
import concourse.tile as tile
from concourse import bass2jax, bass_utils
from concourse.bass_utils import run_bass_kernel_spmd

_NEFF_CACHE = os.path.expanduser("~/.cache/bass_neff")


def _patch_toolchain():
    """Speed up iteration: skip walrus' in-compile BIR simulation (the stock
    neuronx-cc pipeline runs with --enable-birsim=false too) and cache NEFFs
    by BIR hash so identical kernels skip the multi-minute walrus run."""
    if getattr(bass_utils, "_onlstm_patched", False):
        return
    bass_utils._onlstm_patched = True

    orig_run = bass_utils.run_command

    def run_nobirsim(argv, **kw):
        argv = [
            "--enable-birsim=false" if a == "--enable-birsim=true" else a
            for a in argv
        ]
        return orig_run(argv, **kw)

    bass_utils.run_command = run_nobirsim

    orig_compile = bass_utils.compile_bir_kernel

    def cached_compile(bir_json, tmpdir, neff_name="file.neff"):
        raw = bir_json if isinstance(bir_json, bytes) else bir_json.encode()
        try:
            import orjson

            def strip(o):
                if isinstance(o, dict):
                    return {
                        k: strip(v)
                        for k, v in o.items()
                        if k not in ("debug", "debug_info", "ant_debug")
                    }
                if isinstance(o, list):
                    return [strip(v) for v in o]
                return o

            # key on debug-stripped BIR so the cache is path-independent
            key_bytes = orjson.dumps(strip(orjson.loads(raw)))
        except Exception:
            key_bytes = raw
        h = hashlib.sha256(key_bytes).hexdigest()[:32]
        cpath = os.path.join(_NEFF_CACHE, f"{h}.neff")
        dst = os.path.join(tmpdir, neff_name)
        if os.path.exists(cpath):
            shutil.copy(cpath, dst)
            return dst
        out = orig_compile(bir_json, tmpdir, neff_name)
        os.makedirs(_NEFF_CACHE, exist_ok=True)
        tmp = cpath + ".tmp"
        shutil.copy(out, tmp)
        os.replace(tmp, cpath)
        return out

    bass_utils.compile_bir_kernel = cached_compile
    bass2jax.compile_bir_kernel = cached_compile

F32 = mybir.dt.float32
F32R = mybir.dt.float32r
FP16 = mybir.dt.float16
AF = mybir.ActivationFunctionType
ALU = mybir.AluOpType

B, IN, H, CHUNK, NCH = 8192, 1024, 1024, 64, 16
GATE = 4 * H + 2 * NCH  # 4128
NCORES = 8
BC = B // NCORES  # 1024 rows per core
K = IN + H  # 2048 contraction
KT = K // 128  # 16 k-tiles
MT = BC // 128  # 8 batch tiles per core
NT = (4 * H) // 512  # 8 big n-tiles over the 4096 permuted gate cols


def _gate_perm():
    """new_col -> old_col so that gates_new = gates_old[:, perm]."""
    perm = np.empty(GATE, dtype=np.int64)
    for ch in range(NCH):
        base = ch * 4 * CHUNK
        j = np.arange(CHUNK)
        perm[base + 0 * CHUNK + j] = 32 + 1 * H + ch * CHUNK + j  # cell
        perm[base + 1 * CHUNK + j] = 32 + 0 * H + ch * CHUNK + j  # outgate
        perm[base + 2 * CHUNK + j] = 32 + 2 * H + ch * CHUNK + j  # ingate
        perm[base + 3 * CHUNK + j] = 32 + 3 * H + ch * CHUNK + j  # forgetgate
    perm[4 * H : 4 * H + NCH] = np.arange(NCH)  # cin raw
    perm[4 * H + NCH :] = NCH + np.arange(NCH)  # cfg raw
    return perm


def _split_waits(nc):
    """This walrus build caps sem-waits per instruction (1 for NoOp/Drain
    control encodings, 2 for compute/DMA). Move excess waits onto preceding
    single-wait NoOps on the same engine."""
    fn = nc.m.functions[0]
    for blk in fn.blocks:
        newlist = []
        for ins in blk.instructions:
            si = ins.sync_info
            maxw = 1
            if si is not None and len(si.on_wait) > maxw:
                waits = list(si.on_wait)
                while len(waits) > maxw:
                    chunk, waits = waits[:1], waits[1:]
                    nop = mybir.InstNoOp(
                        name=nc.get_next_instruction_name(), ins=[], outs=[]
                    )
                    nop.engine = ins.engine
                    nop.sync_info = mybir.SyncInfo(on_wait=chunk, on_update=[])
                    nc.register_instruction(nop)
                    newlist.append(nop)
                ins.sync_info = mybir.SyncInfo(
                    on_wait=waits, on_update=list(si.on_update)
                )
            newlist.append(ins)
        blk.instructions[:] = newlist


def _build_bass():
    nc = bass.Bass()
    xt = nc.dram_tensor("xt", [K, BC], FP16, kind="ExternalInput")
    wt = nc.dram_tensor("wt", [K, GATE], FP16, kind="ExternalInput")
    biasd = nc.dram_tensor("biasd", [GATE], F32, kind="ExternalInput")
    cxd = nc.dram_tensor("cxd", [BC, H], F32, kind="ExternalInput")
    hyd = nc.dram_tensor("hyd", [BC, H], F32, kind="ExternalOutput")
    cyd = nc.dram_tensor("cyd", [BC, H], F32, kind="ExternalOutput")
    dcfd = nc.dram_tensor("dcfd", [BC], F32, kind="ExternalOutput")
    dcid = nc.dram_tensor("dcid", [BC], F32, kind="ExternalOutput")

    with tile.TileContext(nc) as tc:
        with (
            tc.tile_pool(name="big", bufs=1) as big,
            tc.tile_pool(name="wpool", bufs=10) as wpool,
            tc.tile_pool(name="work", bufs=3) as work,
            tc.tile_pool(name="small", bufs=3) as small,
            tc.tile_pool(name="psum", bufs=8, space="PSUM") as psum,
        ):
            # ---- resident tiles ----
            # All resident loads ride the GpSimd SWDGE queues so the SP HWDGE
            # queue is free for the W stream from t=0. Order: wtail first
            # (phase A blocks on it), then xt k-tiles, then bias/cx (needed
            # only once gate math starts).
            wtail_sb = big.tile([128, KT * 32], FP16)  # cin/cfg weight cols
            for k in range(KT):
                nc.gpsimd.dma_start(
                    out=wtail_sb[:, k * 32 : (k + 1) * 32],
                    in_=wt[k * 128 : (k + 1) * 128, 4 * H :],
                )
            xt_sb = big.tile([128, KT * BC], FP16)  # k-tile k at cols [k*BC,(k+1)*BC)
            for k in range(KT):
                nc.sync.dma_start(
                    out=xt_sb[:, k * BC : (k + 1) * BC],
                    in_=xt[k * 128 : (k + 1) * 128, :],
                )
            bias_sb = big.tile([128, GATE], F32)
            nc.gpsimd.dma_start(
                out=bias_sb,
                in_=bass.AP(tensor=biasd, offset=0, ap=[[0, 128], [1, GATE]]),
            )
            cx_sb = big.tile([128, MT * H], F32)  # m-tile m at cols [m*H,(m+1)*H)
            for m in range(MT):
                nc.gpsimd.dma_start(
                    out=cx_sb[:, m * H : (m + 1) * H],
                    in_=cxd[m * 128 : (m + 1) * 128, :],
                )
            # per-m chunk-gate vectors kept across phase B; icf_all interleaves
            # (cingate-ov, cforgetgate-ov) per chunk so phase B can broadcast
            # both with a single 4D AP
            ov_all = big.tile([128, MT * NCH], F32)
            icf_all = big.tile([128, MT * NCH * 2], F32)
            icf = icf_all.rearrange("p (m c w) -> p m c w", m=MT, c=NCH)

            def lhs(k, m):
                off = k * BC + m * 128
                return xt_sb[:, off : off + 128]

            # ---- phase A: cin/cfg columns, cumsoftmax machinery ----
            for m in range(MT):
                ps_a = psum.tile([128, 32], F32, tag="ps")
                for k in range(KT):
                    nc.tensor.matmul(
                        ps_a,
                        lhs(k, m),
                        wtail_sb[:, k * 32 : (k + 1) * 32],
                        start=(k == 0),
                        stop=(k == KT - 1),
                    )
                raw = small.tile([128, 32], F32)
                nc.vector.tensor_add(raw, ps_a, bias_sb[:, 4 * H :])
                expt = small.tile([128, 32], F32)
                sums = small.tile([128, 2], F32)
                nc.scalar.activation(
                    out=expt[:, :16], in_=raw[:, :16], func=AF.Exp,
                    accum_out=sums[:, 0:1],
                )
                nc.scalar.activation(
                    out=expt[:, 16:], in_=raw[:, 16:], func=AF.Exp,
                    accum_out=sums[:, 1:2],
                )
                rec = small.tile([128, 2], F32)
                nc.vector.reciprocal(out=rec, in_=sums)
                nrec = small.tile([128, 1], F32)
                nc.scalar.mul(out=nrec, in_=rec[:, 0:1], mul=-1.0)
                scan = small.tile([128, 32], F32)
                nc.vector.tensor_tensor_scan(
                    out=scan[:, :16], data0=expt[:, :16], data1=expt[:, :16],
                    initial=0.0, op0=ALU.add, op1=ALU.bypass,
                )
                nc.vector.tensor_tensor_scan(
                    out=scan[:, 16:], data0=expt[:, 16:], data1=expt[:, 16:],
                    initial=0.0, op0=ALU.add, op1=ALU.bypass,
                )
                ci = small.tile([128, NCH], F32)
                nc.vector.tensor_scalar(
                    out=ci, in0=scan[:, :16], scalar1=nrec[:, 0:1], scalar2=1.0,
                    op0=ALU.mult, op1=ALU.add,
                )
                cf = small.tile([128, NCH], F32)
                nc.vector.tensor_scalar_mul(out=cf, in0=scan[:, 16:], scalar1=rec[:, 1:2])
                msl = slice(m * NCH, (m + 1) * NCH)
                nc.vector.tensor_mul(ov_all[:, msl], ci, cf)
                nc.vector.tensor_sub(icf[:, m, :, 0], ci, ov_all[:, msl])
                nc.vector.tensor_sub(icf[:, m, :, 1], cf, ov_all[:, msl])
                # distances
                dsum = small.tile([128, 2], F32)
                nc.vector.reduce_sum(out=dsum[:, 0:1], in_=cf, axis=mybir.AxisListType.X)
                nc.vector.reduce_sum(out=dsum[:, 1:2], in_=ci, axis=mybir.AxisListType.X)
                dout = small.tile([128, 2], F32)
                nc.scalar.activation(
                    out=dout[:, 0:1], in_=dsum[:, 0:1], func=AF.Copy,
                    scale=-1.0 / NCH, bias=1.0,
                )
                nc.scalar.activation(
                    out=dout[:, 1:2], in_=dsum[:, 1:2], func=AF.Copy,
                    scale=1.0 / NCH, bias=0.0,
                )
                nc.gpsimd.dma_start(
                    out=dcfd.rearrange("(m p) -> m p", p=128)[m][:, None],
                    in_=dout[:, 0:1],
                )
                nc.gpsimd.dma_start(
                    out=dcid.rearrange("(m p) -> m p", p=128)[m][:, None],
                    in_=dout[:, 1:2],
                )

            # ---- phase B: main gates, 512 cols (= 2 chunks) per n-tile ----
            cx4 = cx_sb.rearrange("p (m c j) -> p m c j", m=MT, c=NCH)
            cy3 = cyd.rearrange("b (c j) -> b c j", c=NCH)
            hy3 = hyd.rearrange("b (c j) -> b c j", c=NCH)
            for n in range(NT):
                ps_tiles = [psum.tile([128, 512], F32, tag="ps", name=f"ps_{n}_{m}") for m in range(MT)]
                for k in range(KT):
                    wk = wpool.tile([128, 512], FP16, tag="w", name=f"w_{n}_{k}")
                    nc.sync.dma_start(
                        out=wk, in_=wt[k * 128 : (k + 1) * 128, n * 512 : (n + 1) * 512]
                    )
                    for m in range(MT):
                        nc.tensor.matmul(
                            ps_tiles[m],
                            lhs(k, m),
                            wk,
                            start=(k == 0),
                            stop=(k == KT - 1),
                        )
                # evacuate all PSUM banks first: the g-adds are the bank
                # releases, so they must not queue behind full gate chains on
                # DVE's FIFO (else the next n-tile's matmuls stall per bank)
                g_tiles = []
                for m in range(MT):
                    g = work.tile(
                        [128, 2, 256], F32, tag="g", bufs=10, name=f"g_{n}_{m}"
                    )
                    nc.vector.tensor_add(
                        g,
                        ps_tiles[m].rearrange("p (c q) -> p c q", c=2),
                        bias_sb[:, n * 512 : (n + 1) * 512].rearrange(
                            "p (c q) -> p c q", c=2
                        ),
                    )
                    g_tiles.append(g)
                for m in range(MT):
                    g = g_tiles[m]
                    act = work.tile([128, 2, 256], F32, tag="act", name=f"act_{n}_{m}")
                    nc.scalar.activation(
                        out=act[:, :, 64:256], in_=g[:, :, 64:256], func=AF.Sigmoid
                    )
                    nc.scalar.activation(
                        out=act[:, :, 0:64], in_=g[:, :, 0:64], func=AF.Tanh
                    )
                    ch = slice(m * NCH + 2 * n, m * NCH + 2 * n + 2)
                    ovb4 = ov_all[:, ch][:, :, None, None].broadcast_to(
                        [128, 2, 2, 64]
                    )
                    icfb4 = icf[:, m, 2 * n : 2 * n + 2, :][
                        :, :, :, None
                    ].broadcast_to([128, 2, 2, 64])
                    act4 = act[:, :, 128:256].rearrange(
                        "p c (w j) -> p c w j", w=2
                    )  # [.., 0, :]=ingate, [.., 1, :]=forgetgate
                    fi = work.tile([128, 2, 2, 64], F32, tag="fi", name=f"fi_{n}_{m}")
                    nc.vector.tensor_mul(fi, act4, ovb4)
                    nc.vector.tensor_add(fi, fi, icfb4)
                    nc.vector.tensor_mul(fi[:, :, 0, :], fi[:, :, 0, :], act[:, :, 0:64])
                    nc.vector.tensor_mul(
                        fi[:, :, 1, :], fi[:, :, 1, :], cx4[:, m, 2 * n : 2 * n + 2, :]
                    )
                    cyt = work.tile([128, 2, 64], F32, tag="cyt", name=f"cyt_{n}_{m}")
                    nc.vector.tensor_add(cyt, fi[:, :, 0, :], fi[:, :, 1, :])
                    nc.sync.dma_start(
                        out=cy3[m * 128 : (m + 1) * 128, 2 * n : 2 * n + 2, :],
                        in_=cyt,
                    )
                    th = work.tile([128, 2, 64], F32, tag="th", name=f"th_{n}_{m}")
                    nc.scalar.activation(out=th, in_=cyt, func=AF.Tanh)
                    nc.vector.tensor_mul(th, th, act[:, :, 64:128])
                    nc.scalar.dma_start(
                        out=hy3[m * 128 : (m + 1) * 128, 2 * n : 2 * n + 2, :],
                        in_=th,
                    )
    _split_waits(nc)
    return nc


_PERM = _gate_perm()
LAST_RESULT = None


def kernel(x, hx, cx, W_ih, b_ih, W_hh, b_hh):
    x = np.asarray(x, dtype=np.float32)
    hx = np.asarray(hx, dtype=np.float32)
    cx = np.asarray(cx, dtype=np.float32)
    Wcat = np.concatenate(
        [np.asarray(W_ih, np.float32), np.asarray(W_hh, np.float32)], axis=1
    )  # [GATE, K]
    wt_np = np.ascontiguousarray(Wcat[_PERM].T)  # [K, GATE]
    bias_np = np.ascontiguousarray(
        (np.asarray(b_ih, np.float32) + np.asarray(b_hh, np.float32))[_PERM]
    )
    xcatT = np.concatenate([x, hx], axis=1).T  # [K, B]
    cx2 = np.ascontiguousarray(cx.reshape(B, H))
    wt_np = wt_np.astype(np.float16)
    xcatT = np.ascontiguousarray(xcatT.astype(np.float16))

    _patch_toolchain()
    nc = _build_bass()
    in_maps = []
    for c in range(NCORES):
        sl = slice(c * BC, (c + 1) * BC)
        in_maps.append(
            {
                "xt": np.ascontiguousarray(xcatT[:, sl]),
                "wt": wt_np,
                "biasd": bias_np,
                "cxd": cx2[sl],
            }
        )
    global LAST_RESULT
    LAST_RESULT = run_bass_kernel_spmd(nc, in_maps, list(range(NCORES)))
    res = LAST_RESULT.results
    hy = np.concatenate([res[c]["hyd"] for c in range(NCORES)], axis=0)
    cy = np.concatenate([res[c]["cyd"] for c in range(NCORES)], axis=0).reshape(
        B, NCH, CHUNK
    )
    dcf = np.concatenate([res[c]["dcfd"] for c in range(NCORES)], axis=0)
    dci = np.concatenate([res[c]["dcid"] for c in range(NCORES)], axis=0)
    return hy, cy, dcf, dci


# revision 22
# speedup vs baseline: 1.0675x; 1.0195x over previous
"""ON-LSTM cell fused kernel for Trainium2, data-parallel over 8 NeuronCores.

Reference computation (per batch row b):
    gates = x @ W_ih.T + b_ih + hx @ W_hh.T + b_hh          # [B, 4128]
    cin_raw = gates[:, :16]; cfg_raw = gates[:, 16:32]
    rest = gates[:, 32:].reshape(B, 64, 64)
    outgate = sigmoid(rest[:, 0:16]); cell = tanh(rest[:, 16:32])
    ingate = sigmoid(rest[:, 32:48]); forgetgate = sigmoid(rest[:, 48:64])
    cingate = 1 - cumsum(softmax(cin_raw)); cforgetgate = cumsum(softmax(cfg_raw))
    ov = cingate*cforgetgate
    f = forgetgate*ov + (cforgetgate - ov); i = ingate*ov + (cingate - ov)
    cy = f*cx + i*cell; hy = outgate*tanh(cy)
    dcf = 1 - sum(cforgetgate)/16; dci = sum(cingate)/16

Device strategy (per core, 1024 batch rows):
  - Host pre-transposes/concats activations (xT [2048, B]) and weights
    (WT [2048, 4128]) with a gate-column permutation so each chunk's four
    gates are contiguous: new col ch*256 + [cell|out|in|fgt]*64, then
    cin raw at 4096:4112, cfg raw at 4112:4128.
  - gatesT accumulated in PSUM via fp32r matmuls (full PE rate at N=512),
    16 K-tiles, all 8 PSUM banks = 8 batch tiles of 128; each PSUM tile is
    consumed immediately by fused gate math so gates never hit SBUF/DRAM.
  - cumsoftmax via ACT Exp(+accum sum) + DVE tensor_tensor_scan (hw cumsum).
"""

import hashlib
import os
import shutil

import numpy as np

import concourse.bass as bass
import concourse.mybir as mybir
import concourse.tile as tile
from concourse import bass2jax, bass_utils
from concourse.bass_utils import run_bass_kernel_spmd

_NEFF_CACHE = os.path.expanduser("~/.cache/bass_neff")


def _patch_toolchain():
    """Speed up iteration: skip walrus' in-compile BIR simulation (the stock
    neuronx-cc pipeline runs with --enable-birsim=false too) and cache NEFFs
    by BIR hash so identical kernels skip the multi-minute walrus run."""
    if getattr(bass_utils, "_onlstm_patched", False):
        return
    bass_utils._onlstm_patched = True

    orig_run = bass_utils.run_command

    def run_nobirsim(argv, **kw):
        argv = [
            "--enable-birsim=false" if a == "--enable-birsim=true" else a
            for a in argv
        ]
        return orig_run(argv, **kw)

    bass_utils.run_command = run_nobirsim

    orig_compile = bass_utils.compile_bir_kernel

    def cached_compile(bir_json, tmpdir, neff_name="file.neff"):
        raw = bir_json if isinstance(bir_json, bytes) else bir_json.encode()
        try:
            import orjson

            def strip(o):
                if isinstance(o, dict):
                    return {
                        k: strip(v)
                        for k, v in o.items()
                        if k not in ("debug", "debug_info", "ant_debug")
                    }
                if isinstance(o, list):
                    return [strip(v) for v in o]
                return o

            # key on debug-stripped BIR so the cache is path-independent
            key_bytes = orjson.dumps(strip(orjson.loads(raw)))
        except Exception:
            key_bytes = raw
        h = hashlib.sha256(key_bytes).hexdigest()[:32]
        cpath = os.path.join(_NEFF_CACHE, f"{h}.neff")
        dst = os.path.join(tmpdir, neff_name)
        if os.path.exists(cpath):
            shutil.copy(cpath, dst)
            return dst
        out = orig_compile(bir_json, tmpdir, neff_name)
        os.makedirs(_NEFF_CACHE, exist_ok=True)
        tmp = cpath + ".tmp"
        shutil.copy(out, tmp)
        os.replace(tmp, cpath)
        return out

    bass_utils.compile_bir_kernel = cached_compile
    bass2jax.compile_bir_kernel = cached_compile

F32 = mybir.dt.float32
F32R = mybir.dt.float32r
FP16 = mybir.dt.float16
AF = mybir.ActivationFunctionType
ALU = mybir.AluOpType

B, IN, H, CHUNK, NCH = 8192, 1024, 1024, 64, 16
GATE = 4 * H + 2 * NCH  # 4128
NCORES = 8
BC = B // NCORES  # 1024 rows per core
K = IN + H  # 2048 contraction
KT = K // 128  # 16 k-tiles
MT = BC // 128  # 8 batch tiles per core
NT = (4 * H) // 512  # 8 big n-tiles over the 4096 permuted gate cols


def _gate_perm():
    """new_col -> old_col so that gates_new = gates_old[:, perm]."""
    perm = np.empty(GATE, dtype=np.int64)
    for ch in range(NCH):
        base = ch * 4 * CHUNK
        j = np.arange(CHUNK)
        perm[base + 0 * CHUNK + j] = 32 + 1 * H + ch * CHUNK + j  # cell
        perm[base + 1 * CHUNK + j] = 32 + 0 * H + ch * CHUNK + j  # outgate
        perm[base + 2 * CHUNK + j] = 32 + 2 * H + ch * CHUNK + j  # ingate
        perm[base + 3 * CHUNK + j] = 32 + 3 * H + ch * CHUNK + j  # forgetgate
    perm[4 * H : 4 * H + NCH] = np.arange(NCH)  # cin raw
    perm[4 * H + NCH :] = NCH + np.arange(NCH)  # cfg raw
    return perm


def _split_waits(nc):
    """This walrus build caps sem-waits per instruction (1 for NoOp/Drain
    control encodings, 2 for compute/DMA). Move excess waits onto preceding
    single-wait NoOps on the same engine."""
    fn = nc.m.functions[0]
    for blk in fn.blocks:
        newlist = []
        for ins in blk.instructions:
            si = ins.sync_info
            maxw = 1
            if si is not None and len(si.on_wait) > maxw:
                waits = list(si.on_wait)
                while len(waits) > maxw:
                    chunk, waits = waits[:1], waits[1:]
                    nop = mybir.InstNoOp(
                        name=nc.get_next_instruction_name(), ins=[], outs=[]
                    )
                    nop.engine = ins.engine
                    nop.sync_info = mybir.SyncInfo(on_wait=chunk, on_update=[])
                    nc.register_instruction(nop)
                    newlist.append(nop)
                ins.sync_info = mybir.SyncInfo(
                    on_wait=waits, on_update=list(si.on_update)
                )
            newlist.append(ins)
        blk.instructions[:] = newlist


def _build_bass():
    nc = bass.Bass()
    xt = nc.dram_tensor("xt", [K, BC], FP16, kind="ExternalInput")
    wt = nc.dram_tensor("wt", [K, GATE], FP16, kind="ExternalInput")
    biasd = nc.dram_tensor("biasd", [GATE], F32, kind="ExternalInput")
    cxd = nc.dram_tensor("cxd", [BC, H], F32, kind="ExternalInput")
    hyd = nc.dram_tensor("hyd", [BC, H], F32, kind="ExternalOutput")
    cyd = nc.dram_tensor("cyd", [BC, H], F32, kind="ExternalOutput")
    dcfd = nc.dram_tensor("dcfd", [BC], F32, kind="ExternalOutput")
    dcid = nc.dram_tensor("dcid", [BC], F32, kind="ExternalOutput")

    with tile.TileContext(nc) as tc:
        with (
            tc.tile_pool(name="big", bufs=1) as big,
            tc.tile_pool(name="wpool", bufs=10) as wpool,
            tc.tile_pool(name="work", bufs=3) as work,
            tc.tile_pool(name="small", bufs=3) as small,
            tc.tile_pool(name="psum", bufs=8, space="PSUM") as psum,
        ):
            # ---- resident tiles ----
            # All resident loads ride the GpSimd SWDGE queues so the SP HWDGE
            # queue is free for the W stream from t=0. Order: wtail first
            # (phase A blocks on it), then xt k-tiles, then bias/cx (needed
            # only once gate math starts).
            wtail_sb = big.tile([128, KT * 32], FP16)  # cin/cfg weight cols
            for k in range(KT):
                nc.gpsimd.dma_start(
                    out=wtail_sb[:, k * 32 : (k + 1) * 32],
                    in_=wt[k * 128 : (k + 1) * 128, 4 * H :],
                )
            xt_sb = big.tile([128, KT * BC], FP16)  # k-tile k at cols [k*BC,(k+1)*BC)
            for k in range(KT):
                nc.sync.dma_start(
                    out=xt_sb[:, k * BC : (k + 1) * BC],
                    in_=xt[k * 128 : (k + 1) * 128, :],
                )
            bias_sb = big.tile([128, GATE], F32)
            nc.gpsimd.dma_start(
                out=bias_sb,
                in_=bass.AP(tensor=biasd, offset=0, ap=[[0, 128], [1, GATE]]),
            )
            cx_sb = big.tile([128, MT * H], F32)  # m-tile m at cols [m*H,(m+1)*H)
            for m in range(MT):
                nc.gpsimd.dma_start(
                    out=cx_sb[:, m * H : (m + 1) * H],
                    in_=cxd[m * 128 : (m + 1) * 128, :],
                )
            # per-m chunk-gate vectors kept across phase B; icf_all interleaves
            # (cingate-ov, cforgetgate-ov) per chunk so phase B can broadcast
            # both with a single 4D AP
            ov_all = big.tile([128, MT * NCH], F32)
            icf_all = big.tile([128, MT * NCH * 2], F32)
            icf = icf_all.rearrange("p (m c w) -> p m c w", m=MT, c=NCH)

            def lhs(k, m):
                off = k * BC + m * 128
                return xt_sb[:, off : off + 128]

            # ---- phase A: cin/cfg columns, cumsoftmax machinery ----
            for m in range(MT):
                ps_a = psum.tile([128, 32], F32, tag="ps")
                for k in range(KT):
                    nc.tensor.matmul(
                        ps_a,
                        lhs(k, m),
                        wtail_sb[:, k * 32 : (k + 1) * 32],
                        start=(k == 0),
                        stop=(k == KT - 1),
                    )
                raw = small.tile([128, 32], F32)
                nc.vector.tensor_add(raw, ps_a, bias_sb[:, 4 * H :])
                expt = small.tile([128, 32], F32)
                sums = small.tile([128, 2], F32)
                nc.scalar.activation(
                    out=expt[:, :16], in_=raw[:, :16], func=AF.Exp,
                    accum_out=sums[:, 0:1],
                )
                nc.scalar.activation(
                    out=expt[:, 16:], in_=raw[:, 16:], func=AF.Exp,
                    accum_out=sums[:, 1:2],
                )
                rec = small.tile([128, 2], F32)
                nc.vector.reciprocal(out=rec, in_=sums)
                nrec = small.tile([128, 1], F32)
                nc.scalar.mul(out=nrec, in_=rec[:, 0:1], mul=-1.0)
                scan = small.tile([128, 32], F32)
                nc.vector.tensor_tensor_scan(
                    out=scan[:, :16], data0=expt[:, :16], data1=expt[:, :16],
                    initial=0.0, op0=ALU.add, op1=ALU.bypass,
                )
                nc.vector.tensor_tensor_scan(
                    out=scan[:, 16:], data0=expt[:, 16:], data1=expt[:, 16:],
                    initial=0.0, op0=ALU.add, op1=ALU.bypass,
                )
                ci = small.tile([128, NCH], F32)
                nc.vector.tensor_scalar(
                    out=ci, in0=scan[:, :16], scalar1=nrec[:, 0:1], scalar2=1.0,
                    op0=ALU.mult, op1=ALU.add,
                )
                cf = small.tile([128, NCH], F32)
                nc.vector.tensor_scalar_mul(out=cf, in0=scan[:, 16:], scalar1=rec[:, 1:2])
                msl = slice(m * NCH, (m + 1) * NCH)
                nc.vector.tensor_mul(ov_all[:, msl], ci, cf)
                nc.vector.tensor_sub(icf[:, m, :, 0], ci, ov_all[:, msl])
                nc.vector.tensor_sub(icf[:, m, :, 1], cf, ov_all[:, msl])
                # distances
                dsum = small.tile([128, 2], F32)
                nc.vector.reduce_sum(out=dsum[:, 0:1], in_=cf, axis=mybir.AxisListType.X)
                nc.vector.reduce_sum(out=dsum[:, 1:2], in_=ci, axis=mybir.AxisListType.X)
                dout = small.tile([128, 2], F32)
                nc.scalar.activation(
                    out=dout[:, 0:1], in_=dsum[:, 0:1], func=AF.Copy,
                    scale=-1.0 / NCH, bias=1.0,
                )
                nc.scalar.activation(
                    out=dout[:, 1:2], in_=dsum[:, 1:2], func=AF.Copy,
                    scale=1.0 / NCH, bias=0.0,
                )
                nc.gpsimd.dma_start(
                    out=dcfd.rearrange("(m p) -> m p", p=128)[m][:, None],
                    in_=dout[:, 0:1],
                )
                nc.gpsimd.dma_start(
                    out=dcid.rearrange("(m p) -> m p", p=128)[m][:, None],
                    in_=dout[:, 1:2],
                )

            # ---- phase B: main gates, 512 cols (= 2 chunks) per n-tile ----
            cx4 = cx_sb.rearrange("p (m c j) -> p m c j", m=MT, c=NCH)
            cy3 = cyd.rearrange("b (c j) -> b c j", c=NCH)
            hy3 = hyd.rearrange("b (c j) -> b c j", c=NCH)
            for n in range(NT):
                ps_tiles = [psum.tile([128, 512], F32, tag="ps", name=f"ps_{n}_{m}") for m in range(MT)]
                for k in range(KT):
                    wk = wpool.tile([128, 512], FP16, tag="w", name=f"w_{n}_{k}")
                    nc.sync.dma_start(
                        out=wk, in_=wt[k * 128 : (k + 1) * 128, n * 512 : (n + 1) * 512]
                    )
                    for m in range(MT):
                        nc.tensor.matmul(
                            ps_tiles[m],
                            lhs(k, m),
                            wk,
                            start=(k == 0),
                            stop=(k == KT - 1),
                        )
                # evacuate all PSUM banks first: the g-adds are the bank
                # releases, so they must not queue behind full gate chains on
                # DVE's FIFO (else the next n-tile's matmuls stall per bank)
                g_tiles = []
                for m in range(MT):
                    g = work.tile(
                        [128, 2, 256], F32, tag="g", bufs=10, name=f"g_{n}_{m}"
                    )
                    nc.vector.tensor_add(
                        g,
                        ps_tiles[m].rearrange("p (c q) -> p c q", c=2),
                        bias_sb[:, n * 512 : (n + 1) * 512].rearrange(
                            "p (c q) -> p c q", c=2
                        ),
                    )
                    g_tiles.append(g)
                for m in range(MT):
                    g = g_tiles[m]
                    act = work.tile([128, 2, 256], F32, tag="act", name=f"act_{n}_{m}")
                    nc.scalar.activation(
                        out=act[:, :, 64:256], in_=g[:, :, 64:256], func=AF.Sigmoid
                    )
                    nc.scalar.activation(
                        out=act[:, :, 0:64], in_=g[:, :, 0:64], func=AF.Tanh
                    )
                    ch = slice(m * NCH + 2 * n, m * NCH + 2 * n + 2)
                    ovb4 = ov_all[:, ch][:, :, None, None].broadcast_to(
                        [128, 2, 2, 64]
                    )
                    icfb4 = icf[:, m, 2 * n : 2 * n + 2, :][
                        :, :, :, None
                    ].broadcast_to([128, 2, 2, 64])
                    act4 = act[:, :, 128:256].rearrange(
                        "p c (w j) -> p c w j", w=2
                    )  # [.., 0, :]=ingate, [.., 1, :]=forgetgate
                    fi = work.tile([128, 2, 2, 64], F32, tag="fi", name=f"fi_{n}_{m}")
                    nc.vector.tensor_mul(fi, act4, ovb4)
                    nc.vector.tensor_add(fi, fi, icfb4)
                    nc.vector.tensor_mul(fi[:, :, 0, :], fi[:, :, 0, :], act[:, :, 0:64])
                    nc.vector.tensor_mul(
                        fi[:, :, 1, :], fi[:, :, 1, :], cx4[:, m, 2 * n : 2 * n + 2, :]
                    )
                    cyt = work.tile([128, 2, 64], F32, tag="cyt", name=f"cyt_{n}_{m}")
                    nc.vector.tensor_add(cyt, fi[:, :, 0, :], fi[:, :, 1, :])
                    nc.gpsimd.dma_start(
                        out=cy3[m * 128 : (m + 1) * 128, 2 * n : 2 * n + 2, :],
                        in_=cyt,
                    )
                    th = work.tile([128, 2, 64], F32, tag="th", name=f"th_{n}_{m}")
                    nc.scalar.activation(out=th, in_=cyt, func=AF.Tanh)
                    nc.vector.tensor_mul(th, th, act[:, :, 64:128])
                    nc.gpsimd.dma_start(
                        out=hy3[m * 128 : (m + 1) * 128, 2 * n : 2 * n + 2, :],
                        in_=th,
                    )
    _split_waits(nc)
    return nc


_PERM = _gate_perm()
LAST_RESULT = None


def kernel(x, hx, cx, W_ih, b_ih, W_hh, b_hh):
    x = np.asarray(x, dtype=np.float32)
    hx = np.asarray(hx, dtype=np.float32)
    cx = np.asarray(cx, dtype=np.float32)
    Wcat = np.concatenate(
        [np.asarray(W_ih, np.float32), np.asarray(W_hh, np.float32)], axis=1
    )  # [GATE, K]
    wt_np = np.ascontiguousarray(Wcat[_PERM].T)  # [K, GATE]
    bias_np = np.ascontiguousarray(
        (np.asarray(b_ih, np.float32) + np.asarray(b_hh, np.float32))[_PERM]
    )
    xcatT = np.concatenate([x, hx], axis=1).T  # [K, B]
    cx2 = np.ascontiguousarray(cx.reshape(B, H))
    wt_np = wt_np.astype(np.float16)
    xcatT = np.ascontiguousarray(xcatT.astype(np.float16))

    _patch_toolchain()
    nc = _build_bass()
    in_maps = []
    for c in range(NCORES):
        sl = slice(c * BC, (c + 1) * BC)
        in_maps.append(
            {
                "xt": np.ascontiguousarray(xcatT[:, sl]),
                "wt": wt_np,
                "biasd": bias_np,
                "cxd": cx2[sl],
            }
        )
    global LAST_RESULT
    LAST_RESULT = run_bass_kernel_spmd(nc, in_maps, list(range(NCORES)))
    res = LAST_RESULT.results
    hy = np.concatenate([res[c]["hyd"] for c in range(NCORES)], axis=0)
    cy = np.concatenate([res[c]["cyd"] for c in range(NCORES)], axis=0).reshape(
        B, NCH, CHUNK
    )
    dcf = np.concatenate([res[c]["dcfd"] for c in range(NCORES)], axis=0)
    dci = np.concatenate([res[c]["dcid"] for c in range(NCORES)], axis=0)
    return hy, cy, dcf, dci


# revision 23
# speedup vs baseline: 1.1986x; 1.1228x over previous
"""ON-LSTM cell fused kernel for Trainium2, data-parallel over 8 NeuronCores.

Reference computation (per batch row b):
    gates = x @ W_ih.T + b_ih + hx @ W_hh.T + b_hh          # [B, 4128]
    cin_raw = gates[:, :16]; cfg_raw = gates[:, 16:32]
    rest = gates[:, 32:].reshape(B, 64, 64)
    outgate = sigmoid(rest[:, 0:16]); cell = tanh(rest[:, 16:32])
    ingate = sigmoid(rest[:, 32:48]); forgetgate = sigmoid(rest[:, 48:64])
    cingate = 1 - cumsum(softmax(cin_raw)); cforgetgate = cumsum(softmax(cfg_raw))
    ov = cingate*cforgetgate
    f = forgetgate*ov + (cforgetgate - ov); i = ingate*ov + (cingate - ov)
    cy = f*cx + i*cell; hy = outgate*tanh(cy)
    dcf = 1 - sum(cforgetgate)/16; dci = sum(cingate)/16

Device strategy (per core, 1024 batch rows):
  - Host pre-transposes/concats activations (xT [2048, B]) and weights
    (WT [2048, 4128]) with a gate-column permutation so each chunk's four
    gates are contiguous: new col ch*256 + [cell|out|in|fgt]*64, then
    cin raw at 4096:4112, cfg raw at 4112:4128.
  - gatesT accumulated in PSUM via fp32r matmuls (full PE rate at N=512),
    16 K-tiles, all 8 PSUM banks = 8 batch tiles of 128; each PSUM tile is
    consumed immediately by fused gate math so gates never hit SBUF/DRAM.
  - cumsoftmax via ACT Exp(+accum sum) + DVE tensor_tensor_scan (hw cumsum).
"""

import hashlib
import os
import shutil

import numpy as np

import concourse.bass as bass
import concourse.mybir as mybir
import concourse.tile as tile
from concourse import bass2jax, bass_utils
from concourse.bass_utils import run_bass_kernel_spmd

_NEFF_CACHE = os.path.expanduser("~/.cache/bass_neff")


def _patch_toolchain():
    """Speed up iteration: skip walrus' in-compile BIR simulation (the stock
    neuronx-cc pipeline runs with --enable-birsim=false too) and cache NEFFs
    by BIR hash so identical kernels skip the multi-minute walrus run."""
    if getattr(bass_utils, "_onlstm_patched", False):
        return
    bass_utils._onlstm_patched = True

    orig_run = bass_utils.run_command

    def run_nobirsim(argv, **kw):
        argv = [
            "--enable-birsim=false" if a == "--enable-birsim=true" else a
            for a in argv
        ]
        return orig_run(argv, **kw)

    bass_utils.run_command = run_nobirsim

    orig_compile = bass_utils.compile_bir_kernel

    def cached_compile(bir_json, tmpdir, neff_name="file.neff"):
        raw = bir_json if isinstance(bir_json, bytes) else bir_json.encode()
        try:
            import orjson

            def strip(o):
                if isinstance(o, dict):
                    return {
                        k: strip(v)
                        for k, v in o.items()
                        if k not in ("debug", "debug_info", "ant_debug")
                    }
                if isinstance(o, list):
                    return [strip(v) for v in o]
                return o

            # key on debug-stripped BIR so the cache is path-independent
            key_bytes = orjson.dumps(strip(orjson.loads(raw)))
        except Exception:
            key_bytes = raw
        h = hashlib.sha256(key_bytes).hexdigest()[:32]
        cpath = os.path.join(_NEFF_CACHE, f"{h}.neff")
        dst = os.path.join(tmpdir, neff_name)
        if os.path.exists(cpath):
            shutil.copy(cpath, dst)
            return dst
        out = orig_compile(bir_json, tmpdir, neff_name)
        os.makedirs(_NEFF_CACHE, exist_ok=True)
        tmp = cpath + ".tmp"
        shutil.copy(out, tmp)
        os.replace(tmp, cpath)
        return out

    bass_utils.compile_bir_kernel = cached_compile
    bass2jax.compile_bir_kernel = cached_compile

F32 = mybir.dt.float32
F32R = mybir.dt.float32r
FP16 = mybir.dt.float16
AF = mybir.ActivationFunctionType
ALU = mybir.AluOpType

B, IN, H, CHUNK, NCH = 8192, 1024, 1024, 64, 16
GATE = 4 * H + 2 * NCH  # 4128
NCORES = 8
BC = B // NCORES  # 1024 rows per core
K = IN + H  # 2048 contraction
KT = K // 128  # 16 k-tiles
MT = BC // 128  # 8 batch tiles per core
NT = (4 * H) // 512  # 8 big n-tiles over the 4096 permuted gate cols


def _gate_perm():
    """new_col -> old_col so that gates_new = gates_old[:, perm]."""
    perm = np.empty(GATE, dtype=np.int64)
    for ch in range(NCH):
        base = ch * 4 * CHUNK
        j = np.arange(CHUNK)
        perm[base + 0 * CHUNK + j] = 32 + 1 * H + ch * CHUNK + j  # cell
        perm[base + 1 * CHUNK + j] = 32 + 0 * H + ch * CHUNK + j  # outgate
        perm[base + 2 * CHUNK + j] = 32 + 2 * H + ch * CHUNK + j  # ingate
        perm[base + 3 * CHUNK + j] = 32 + 3 * H + ch * CHUNK + j  # forgetgate
    perm[4 * H : 4 * H + NCH] = np.arange(NCH)  # cin raw
    perm[4 * H + NCH :] = NCH + np.arange(NCH)  # cfg raw
    return perm


def _split_waits(nc):
    """This walrus build caps sem-waits per instruction (1 for NoOp/Drain
    control encodings, 2 for compute/DMA). Move excess waits onto preceding
    single-wait NoOps on the same engine."""
    fn = nc.m.functions[0]
    for blk in fn.blocks:
        newlist = []
        for ins in blk.instructions:
            si = ins.sync_info
            maxw = 1
            if si is not None and len(si.on_wait) > maxw:
                waits = list(si.on_wait)
                while len(waits) > maxw:
                    chunk, waits = waits[:1], waits[1:]
                    nop = mybir.InstNoOp(
                        name=nc.get_next_instruction_name(), ins=[], outs=[]
                    )
                    nop.engine = ins.engine
                    nop.sync_info = mybir.SyncInfo(on_wait=chunk, on_update=[])
                    nc.register_instruction(nop)
                    newlist.append(nop)
                ins.sync_info = mybir.SyncInfo(
                    on_wait=waits, on_update=list(si.on_update)
                )
            newlist.append(ins)
        blk.instructions[:] = newlist


def _build_bass():
    nc = bass.Bass()
    xt = nc.dram_tensor("xt", [K, BC], FP16, kind="ExternalInput")
    wt = nc.dram_tensor("wt", [K, GATE], FP16, kind="ExternalInput")
    biasd = nc.dram_tensor("biasd", [GATE], F32, kind="ExternalInput")
    cxd = nc.dram_tensor("cxd", [BC, H], F32, kind="ExternalInput")
    hyd = nc.dram_tensor("hyd", [BC, H], F32, kind="ExternalOutput")
    cyd = nc.dram_tensor("cyd", [BC, H], F32, kind="ExternalOutput")
    dcfd = nc.dram_tensor("dcfd", [BC], F32, kind="ExternalOutput")
    dcid = nc.dram_tensor("dcid", [BC], F32, kind="ExternalOutput")

    with tile.TileContext(nc) as tc:
        with (
            tc.tile_pool(name="big", bufs=1) as big,
            tc.tile_pool(name="wpool", bufs=10) as wpool,
            tc.tile_pool(name="work", bufs=3) as work,
            tc.tile_pool(name="small", bufs=3) as small,
            tc.tile_pool(name="psum", bufs=8, space="PSUM") as psum,
        ):
            # ---- resident tiles ----
            # All resident loads ride the GpSimd SWDGE queues so the SP HWDGE
            # queue is free for the W stream from t=0. Order: wtail first
            # (phase A blocks on it), then xt k-tiles, then bias/cx (needed
            # only once gate math starts).
            wtail_sb = big.tile([128, KT * 32], FP16)  # cin/cfg weight cols
            for k in range(KT):
                nc.gpsimd.dma_start(
                    out=wtail_sb[:, k * 32 : (k + 1) * 32],
                    in_=wt[k * 128 : (k + 1) * 128, 4 * H :],
                )
            xt_sb = big.tile([128, KT * BC], FP16)  # k-tile k at cols [k*BC,(k+1)*BC)
            for k in range(KT):
                nc.sync.dma_start(
                    out=xt_sb[:, k * BC : (k + 1) * BC],
                    in_=xt[k * 128 : (k + 1) * 128, :],
                )
            bias_sb = big.tile([128, GATE], F32)
            nc.gpsimd.dma_start(
                out=bias_sb,
                in_=bass.AP(tensor=biasd, offset=0, ap=[[0, 128], [1, GATE]]),
            )
            cx_sb = big.tile([128, MT * H], F32)  # m-tile m at cols [m*H,(m+1)*H)
            for m in range(MT):
                nc.gpsimd.dma_start(
                    out=cx_sb[:, m * H : (m + 1) * H],
                    in_=cxd[m * 128 : (m + 1) * 128, :],
                )
            # per-m chunk-gate vectors kept across phase B; icf_all interleaves
            # (cingate-ov, cforgetgate-ov) per chunk so phase B can broadcast
            # both with a single 4D AP
            ov_all = big.tile([128, MT * NCH], F32)
            icf_all = big.tile([128, MT * NCH * 2], F32)
            icf = icf_all.rearrange("p (m c w) -> p m c w", m=MT, c=NCH)

            def lhs(k, m):
                off = k * BC + m * 128
                return xt_sb[:, off : off + 128]

            # ---- phase A: cin/cfg columns, cumsoftmax machinery ----
            for m in range(MT):
                ps_a = psum.tile([128, 32], F32, tag="ps")
                for k in range(KT):
                    nc.tensor.matmul(
                        ps_a,
                        lhs(k, m),
                        wtail_sb[:, k * 32 : (k + 1) * 32],
                        start=(k == 0),
                        stop=(k == KT - 1),
                    )
                raw = small.tile([128, 32], F32)
                nc.vector.tensor_add(raw, ps_a, bias_sb[:, 4 * H :])
                expt = small.tile([128, 32], F32)
                sums = small.tile([128, 2], F32)
                nc.scalar.activation(
                    out=expt[:, :16], in_=raw[:, :16], func=AF.Exp,
                    accum_out=sums[:, 0:1],
                )
                nc.scalar.activation(
                    out=expt[:, 16:], in_=raw[:, 16:], func=AF.Exp,
                    accum_out=sums[:, 1:2],
                )
                rec = small.tile([128, 2], F32)
                nc.vector.reciprocal(out=rec, in_=sums)
                nrec = small.tile([128, 1], F32)
                nc.scalar.mul(out=nrec, in_=rec[:, 0:1], mul=-1.0)
                scan = small.tile([128, 32], F32)
                nc.vector.tensor_tensor_scan(
                    out=scan[:, :16], data0=expt[:, :16], data1=expt[:, :16],
                    initial=0.0, op0=ALU.add, op1=ALU.bypass,
                )
                nc.vector.tensor_tensor_scan(
                    out=scan[:, 16:], data0=expt[:, 16:], data1=expt[:, 16:],
                    initial=0.0, op0=ALU.add, op1=ALU.bypass,
                )
                ci = small.tile([128, NCH], F32)
                nc.vector.tensor_scalar(
                    out=ci, in0=scan[:, :16], scalar1=nrec[:, 0:1], scalar2=1.0,
                    op0=ALU.mult, op1=ALU.add,
                )
                cf = small.tile([128, NCH], F32)
                nc.vector.tensor_scalar_mul(out=cf, in0=scan[:, 16:], scalar1=rec[:, 1:2])
                msl = slice(m * NCH, (m + 1) * NCH)
                nc.vector.tensor_mul(ov_all[:, msl], ci, cf)
                nc.vector.tensor_sub(icf[:, m, :, 0], ci, ov_all[:, msl])
                nc.vector.tensor_sub(icf[:, m, :, 1], cf, ov_all[:, msl])
                # distances
                dsum = small.tile([128, 2], F32)
                nc.vector.reduce_sum(out=dsum[:, 0:1], in_=cf, axis=mybir.AxisListType.X)
                nc.vector.reduce_sum(out=dsum[:, 1:2], in_=ci, axis=mybir.AxisListType.X)
                dout = small.tile([128, 2], F32)
                nc.scalar.activation(
                    out=dout[:, 0:1], in_=dsum[:, 0:1], func=AF.Copy,
                    scale=-1.0 / NCH, bias=1.0,
                )
                nc.scalar.activation(
                    out=dout[:, 1:2], in_=dsum[:, 1:2], func=AF.Copy,
                    scale=1.0 / NCH, bias=0.0,
                )
                nc.gpsimd.dma_start(
                    out=dcfd.rearrange("(m p) -> m p", p=128)[m][:, None],
                    in_=dout[:, 0:1],
                )
                nc.gpsimd.dma_start(
                    out=dcid.rearrange("(m p) -> m p", p=128)[m][:, None],
                    in_=dout[:, 1:2],
                )

            # ---- phase B: main gates, 512 cols (= 2 chunks) per n-tile ----
            cx4 = cx_sb.rearrange("p (m c j) -> p m c j", m=MT, c=NCH)
            cy3 = cyd.rearrange("b (c j) -> b c j", c=NCH)
            hy3 = hyd.rearrange("b (c j) -> b c j", c=NCH)
            for n in range(NT):
                ps_tiles = [psum.tile([128, 512], F32, tag="ps", name=f"ps_{n}_{m}") for m in range(MT)]
                for k in range(KT):
                    wk = wpool.tile([128, 512], FP16, tag="w", name=f"w_{n}_{k}")
                    nc.sync.dma_start(
                        out=wk, in_=wt[k * 128 : (k + 1) * 128, n * 512 : (n + 1) * 512]
                    )
                    for m in range(MT):
                        nc.tensor.matmul(
                            ps_tiles[m],
                            lhs(k, m),
                            wk,
                            start=(k == 0),
                            stop=(k == KT - 1),
                        )
                # evacuate all PSUM banks first: the g-adds are the bank
                # releases, so they must not queue behind full gate chains on
                # DVE's FIFO (else the next n-tile's matmuls stall per bank)
                g_tiles = []
                for m in range(MT):
                    g = work.tile(
                        [128, 2, 256], F32, tag="g", bufs=10, name=f"g_{n}_{m}"
                    )
                    nc.vector.tensor_add(
                        g,
                        ps_tiles[m].rearrange("p (c q) -> p c q", c=2),
                        bias_sb[:, n * 512 : (n + 1) * 512].rearrange(
                            "p (c q) -> p c q", c=2
                        ),
                    )
                    g_tiles.append(g)
                for m in range(MT):
                    g = g_tiles[m]
                    act = work.tile([128, 2, 256], F32, tag="act", name=f"act_{n}_{m}")
                    nc.scalar.activation(
                        out=act[:, :, 64:256], in_=g[:, :, 64:256], func=AF.Sigmoid
                    )
                    nc.scalar.activation(
                        out=act[:, :, 0:64], in_=g[:, :, 0:64], func=AF.Tanh
                    )
                    ch = slice(m * NCH + 2 * n, m * NCH + 2 * n + 2)
                    ovb = ov_all[:, ch][:, :, None].broadcast_to([128, 2, 64])
                    cib = icf[:, m, 2 * n : 2 * n + 2, 0][:, :, None].broadcast_to(
                        [128, 2, 64]
                    )
                    cfb = icf[:, m, 2 * n : 2 * n + 2, 1][:, :, None].broadcast_to(
                        [128, 2, 64]
                    )
                    f1 = work.tile([128, 2, 64], F32, tag="f1", name=f"f1_{n}_{m}")
                    nc.vector.tensor_mul(f1, act[:, :, 192:256], ovb)
                    nc.vector.tensor_add(f1, f1, cfb)
                    i1 = work.tile([128, 2, 64], F32, tag="i1", name=f"i1_{n}_{m}")
                    nc.vector.tensor_mul(i1, act[:, :, 128:192], ovb)
                    nc.vector.tensor_add(i1, i1, cib)
                    cyt = work.tile([128, 2, 64], F32, tag="cyt", name=f"cyt_{n}_{m}")
                    nc.vector.tensor_mul(cyt, f1, cx4[:, m, 2 * n : 2 * n + 2, :])
                    nc.vector.tensor_mul(i1, i1, act[:, :, 0:64])
                    nc.vector.tensor_add(cyt, cyt, i1)
                    nc.gpsimd.dma_start(
                        out=cy3[m * 128 : (m + 1) * 128, 2 * n : 2 * n + 2, :],
                        in_=cyt,
                    )
                    th = work.tile([128, 2, 64], F32, tag="th", name=f"th_{n}_{m}")
                    nc.scalar.activation(out=th, in_=cyt, func=AF.Tanh)
                    nc.vector.tensor_mul(th, th, act[:, :, 64:128])
                    nc.gpsimd.dma_start(
                        out=hy3[m * 128 : (m + 1) * 128, 2 * n : 2 * n + 2, :],
                        in_=th,
                    )
    _split_waits(nc)
    return nc


_PERM = _gate_perm()
LAST_RESULT = None


def kernel(x, hx, cx, W_ih, b_ih, W_hh, b_hh):
    x = np.asarray(x, dtype=np.float32)
    hx = np.asarray(hx, dtype=np.float32)
    cx = np.asarray(cx, dtype=np.float32)
    Wcat = np.concatenate(
        [np.asarray(W_ih, np.float32), np.asarray(W_hh, np.float32)], axis=1
    )  # [GATE, K]
    wt_np = np.ascontiguousarray(Wcat[_PERM].T)  # [K, GATE]
    bias_np = np.ascontiguousarray(
        (np.asarray(b_ih, np.float32) + np.asarray(b_hh, np.float32))[_PERM]
    )
    xcatT = np.concatenate([x, hx], axis=1).T  # [K, B]
    cx2 = np.ascontiguousarray(cx.reshape(B, H))
    wt_np = wt_np.astype(np.float16)
    xcatT = np.ascontiguousarray(xcatT.astype(np.float16))

    _patch_toolchain()
    nc = _build_bass()
    in_maps = []
    for c in range(NCORES):
        sl = slice(c * BC, (c + 1) * BC)
        in_maps.append(
            {
                "xt": np.ascontiguousarray(xcatT[:, sl]),
                "wt": wt_np,
                "biasd": bias_np,
                "cxd": cx2[sl],
            }
        )
    global LAST_RESULT
    LAST_RESULT = run_bass_kernel_spmd(nc, in_maps, list(range(NCORES)))
    res = LAST_RESULT.results
    hy = np.concatenate([res[c]["hyd"] for c in range(NCORES)], axis=0)
    cy = np.concatenate([res[c]["cyd"] for c in range(NCORES)], axis=0).reshape(
        B, NCH, CHUNK
    )
    dcf = np.concatenate([res[c]["dcfd"] for c in range(NCORES)], axis=0)
    dci = np.concatenate([res[c]["dcid"] for c in range(NCORES)], axis=0)
    return hy, cy, dcf, dci
